# revision 5
# baseline (speedup 1.0000x reference)
import sys, functools

if "/opt/trn_rl_repo" not in sys.path:
    sys.path.insert(0, "/opt/trn_rl_repo")

import numpy as np
import ml_dtypes

from concourse import bacc
import concourse.bass as bass
import concourse.mybir as mybir
import concourse.tile as tile
from concourse.bass_utils import run_bass_kernel_spmd

BF16 = mybir.dt.bfloat16
F32 = mybir.dt.float32
AF = mybir.ActivationFunctionType
ALU = mybir.AluOpType
AX = mybir.AxisListType

S, D, HD, H, MLPH = 4096, 3072, 128, 24, 9216
NCORES = 8
HL = H // NCORES            # 3 heads per core
FQ = HL * HD                # 384
FM = MLPH // NCORES         # 1152
FMT = FM // 128             # 9 f-tiles of mlp hidden
FAB = 2 * FM                # 2304 (a/b interleaved in 128-col pairs)
FQKV = 3 * FQ               # 1152
NCOL = D // NCORES          # 384 output cols per core
FO = FQ + FM                # 1536 rows of fused output weight
WOT = FO // 128             # 12 contraction tiles of output proj
EPS = 1e-6
SCH = 1024                  # s-chunk for projection phase
NSC = S // SCH              # 4
KT = D // 128               # 24 contraction tiles of input proj
NKT = S // 128              # 32 k tiles in attention
QC = 512                    # q-chunk for attention/output phase
NQC = S // QC               # 8
NRS = 4                     # number of ReduceScatter chunks
RSW = S // NRS              # 1024 s-columns per RS chunk
SUBRS = NQC // NRS          # q-chunks per RS chunk

LAST_RESULT = None          # test.py introspection


def _to_bf16(a):
    """Fast round-to-nearest f32 -> bf16."""
    a = np.ascontiguousarray(a, np.float32)
    u = a.view(np.uint32)
    r = ((u >> 16) & 1) + np.uint32(0x7FFF)
    return ((u + r) >> 16).astype(np.uint16).view(ml_dtypes.bfloat16)


@functools.lru_cache(maxsize=1)
def _build():
    nc = bacc.Bacc(
        "TRN2",
        target_bir_lowering=False,
        debug=False,
        enable_asserts=False,
        num_devices=NCORES,
    )
    x = nc.dram_tensor("x", [S, D], BF16, kind="ExternalInput").ap()
    w1q = nc.dram_tensor("w1qkv", [D, FQKV], BF16, kind="ExternalInput").ap()
    w1ab = nc.dram_tensor("w1ab", [D, FAB], BF16, kind="ExternalInput").ap()
    c2q = nc.dram_tensor("c2q", [1, FQKV], BF16, kind="ExternalInput").ap()
    c2ab = nc.dram_tensor("c2ab", [128, 2 * FMT], F32, kind="ExternalInput").ap()
    cosb = nc.dram_tensor("cosb", [S, HD], BF16, kind="ExternalInput").ap()
    sinb = nc.dram_tensor("sinb", [S, HD], BF16, kind="ExternalInput").ap()
    qwb = nc.dram_tensor("qwb", [128, HD], F32, kind="ExternalInput").ap()
    kwb = nc.dram_tensor("kwb", [128, HD], F32, kind="ExternalInput").ap()
    wout = nc.dram_tensor("wout", [FO, D], BF16, kind="ExternalInput").ap()
    resT = nc.dram_tensor("resT", [NCOL, S], F32, kind="ExternalInput").ap()
    out_t = nc.dram_tensor("out", [NCOL, S], F32, kind="ExternalOutput").ap()

    rg = [list(range(NCORES))]

    with tile.TileContext(nc) as tc:
        with (
            tc.tile_pool(name="const", bufs=1) as const,
            tc.tile_pool(name="dram", bufs=1, space="DRAM") as dram,
        ):
            ones_p = const.tile([1, 128], BF16)
            nc.vector.memset(ones_p, 1.0)
            ones128 = const.tile([128, 128], BF16)
            nc.vector.memset(ones128, 1.0)
            eps_sb = const.tile([128, 1], F32)
            nc.vector.memset(eps_sb, EPS)
            qwb_sb = const.tile([128, HD], F32)
            nc.sync.dma_start(qwb_sb, qwb)
            kwb_sb = const.tile([128, HD], F32)
            nc.sync.dma_start(kwb_sb, kwb)
            c2q_sb = const.tile([1, FQKV], BF16)
            nc.sync.dma_start(c2q_sb, c2q)
            c2ab_sb = const.tile([128, 2 * FMT], F32)
            nc.sync.dma_start(c2ab_sb, c2ab)

            tti_d = dram.tile([S, D], BF16)     # normalized x, s-major
            qr_d = dram.tile([S, FQ], BF16)     # rope(q), s-major
            kr_d = dram.tile([S, FQ], BF16)
            v_d = dram.tile([S, FQ], BF16)
            m_f = dram.tile([FM, S], BF16)      # swiglu output, f-major
            pall = [dram.tile([D, RSW], BF16, tag=f"pall{i}", name=f"pall{i}")
                    for i in range(NRS)]
            rs_out = [dram.tile([NCOL, RSW], BF16, tag=f"rso{i}", name=f"rso{i}")
                      for i in range(NRS)]

            # ---------------- Phase P: LN + transpose + QKV/MLP projection ----
            with (
                tc.tile_pool(name="w1qp", bufs=1) as w1qp,
                tc.tile_pool(name="xp", bufs=2) as xp,
                tc.tile_pool(name="tp", bufs=3) as tp,
                tc.tile_pool(name="ttp", bufs=1) as ttp,
                tc.tile_pool(name="w1s", bufs=6) as w1s,
                tc.tile_pool(name="smal", bufs=12) as smal,
                tc.tile_pool(name="stg", bufs=3) as stg,
                tc.tile_pool(name="abp", bufs=3) as abp,
                tc.tile_pool(name="psab", bufs=1, space="PSUM") as psab,
                tc.tile_pool(name="psq", bufs=2, space="PSUM") as psq,
            ):
                # resident QKV weight: [dm_part, kt, 1152]; loaded in 4 pieces
                # interleaved with the first LN tiles to not block the x loads
                w1q_sb = w1qp.tile([128, KT, FQKV], BF16)
                w1q_r = w1q.rearrange("(kt p) f -> p kt f", p=128)

                for sc in range(NSC):
                    tT = ttp.tile([128, KT, SCH], BF16, tag="tT")
                    for ss in range(8):
                        s0 = sc * SCH + ss * 128
                        xt = xp.tile([128, D], BF16, tag="x")
                        nc.sync.dma_start(xt, x[s0 : s0 + 128, :])
                        if sc == 0 and ss < 4:
                            k6 = KT // 4
                            nc.sync.dma_start(
                                w1q_sb[:, ss * k6 : (ss + 1) * k6, :],
                                w1q_r[:, ss * k6 : (ss + 1) * k6, :])
                        s1 = smal.tile([128, 1], F32, tag="s1")
                        nc.vector.reduce_sum(s1, xt, axis=AX.X)
                        nmu = smal.tile([128, 1], F32, tag="nmu")
                        nc.scalar.mul(nmu, s1, -1.0 / D)
                        sqs = tp.tile([128, D], BF16, tag="sq", bufs=2)
                        v2 = smal.tile([128, 1], F32, tag="v2")
                        nc.scalar.activation(sqs, xt, AF.Square, bias=nmu, scale=1.0,
                                             accum_out=v2)
                        std = smal.tile([128, 1], F32, tag="std")
                        nc.scalar.activation(std, v2, AF.Sqrt, bias=eps_sb, scale=1.0 / D)
                        rstd = smal.tile([128, 1], F32, tag="rstd")
                        nc.vector.reciprocal(rstd, std)
                        nmr = smal.tile([128, 1], F32, tag="nmr")
                        nc.vector.tensor_mul(out=nmr, in0=nmu, in1=rstd)
                        tti = tp.tile([128, D], BF16, tag="t")
                        nc.scalar.activation(tti, xt, AF.Identity, bias=nmr, scale=rstd)
                        # round-trip through DRAM + XBAR dma transpose -> tT
                        nc.sync.dma_start(tti_d[s0 : s0 + 128, :], tti)
                        nc.scalar.dma_start_transpose(
                            tT[:, :, ss * 128 : (ss + 1) * 128],
                            tti_d[s0 : s0 + 128, :])

                    # --- QKV pass (s-major): one psum bank per j ---
                    for ss in range(8):
                        s0 = sc * SCH + ss * 128
                        sl = ss * 128
                        cos_t = stg.tile([128, HD], BF16, tag="cos")
                        nc.scalar.dma_start(cos_t, cosb[s0 : s0 + 128, :])
                        sin_t = stg.tile([128, HD], BF16, tag="sin")
                        nc.scalar.dma_start(sin_t, sinb[s0 : s0 + 128, :])
                        s2_ = sin_t.rearrange("p (x two) -> p x two", two=2)
                        for j in range(3):
                            pq = psq.tile([128, 512], F32, tag="pqkv")
                            for kt in range(KT):
                                nc.tensor.matmul(
                                    pq[:, :FQ],
                                    tT[:, kt, sl : sl + 128],
                                    w1q_sb[:, kt, j * FQ : (j + 1) * FQ],
                                    start=(kt == 0), stop=False)
                            nc.tensor.matmul(
                                pq[:, :FQ], ones_p,
                                c2q_sb[:, j * FQ : (j + 1) * FQ],
                                start=False, stop=True)
                            if j == 2:
                                vstg = stg.tile([128, FQ], BF16, tag="vst", bufs=2)
                                nc.scalar.copy(vstg, pq[:, :FQ])
                                nc.sync.dma_start(v_d[s0 : s0 + 128, :], vstg)
                                continue
                            wb = qwb_sb if j == 0 else kwb_sb
                            qn = stg.tile([128, FQ], BF16, tag=f"qn{j}")
                            qrr = stg.tile([128, FQ], BF16, tag=f"qr{j}")
                            tmp = stg.tile([128, FQ], BF16, tag=f"tm{j}")
                            for hh in range(HL):
                                blk = pq[:, hh * HD : (hh + 1) * HD]
                                ssq = smal.tile([128, 1], F32, tag="ssq")
                                sq2 = stg.tile([128, HD], F32, tag="sq2")
                                nc.scalar.activation(sq2, blk, AF.Square, accum_out=ssq)
                                sstd = smal.tile([128, 1], F32, tag="sstd")
                                nc.scalar.activation(sstd, ssq, AF.Sqrt,
                                                     bias=eps_sb, scale=1.0 / HD)
                                rst = smal.tile([128, 1], F32, tag="rst")
                                nc.vector.reciprocal(rst, sstd)
                                qnb = qn[:, hh * HD : (hh + 1) * HD]
                                nc.vector.scalar_tensor_tensor(
                                    qnb, blk, rst, wb, ALU.mult, ALU.mult)
                                q3 = qnb.rearrange("p (x two) -> p x two", two=2)
                                t3 = tmp[:, hh * HD : (hh + 1) * HD].rearrange(
                                    "p (x two) -> p x two", two=2)
                                nc.vector.tensor_mul(out=t3[:, :, 0], in0=q3[:, :, 1],
                                                     in1=s2_[:, :, 0])
                                nc.vector.tensor_mul(out=t3[:, :, 1], in0=q3[:, :, 0],
                                                     in1=s2_[:, :, 1])
                                nc.vector.tensor_mul(
                                    out=qrr[:, hh * HD : (hh + 1) * HD],
                                    in0=qnb, in1=cos_t)
                            nc.vector.tensor_add(out=qrr, in0=qrr, in1=tmp)
                            dst = qr_d if j == 0 else kr_d
                            nc.sync.dma_start(dst[s0 : s0 + 128, :], qrr)

                    # --- a/b (f-major) + SwiGLU ---
                    for fb in range(FMT):
                        pa = psab.tile([128, 2, 2, 512], F32, tag="pab")
                        for kt in range(KT):
                            wt = w1s.tile([128, 256], BF16, tag="w1ab")
                            nc.sync.dma_start(
                                wt, w1ab[kt * 128 : (kt + 1) * 128,
                                         fb * 256 : (fb + 1) * 256])
                            for f2 in range(2):
                                for sh in range(2):
                                    nc.tensor.matmul(
                                        pa[:, f2, sh, :],
                                        wt[:, f2 * 128 : (f2 + 1) * 128],
                                        tT[:, kt, sh * 512 : (sh + 1) * 512],
                                        start=(kt == 0), stop=(kt == KT - 1))
                        a_sb = abp.tile([128, 2, 512], BF16, tag="asb")
                        nc.scalar.activation(a_sb, pa[:, 0], AF.Silu,
                                             bias=c2ab_sb[:, 2 * fb : 2 * fb + 1])
                        m_sb = abp.tile([128, 2, 512], BF16, tag="msb")
                        nc.vector.scalar_tensor_tensor(
                            m_sb, pa[:, 1], c2ab_sb[:, 2 * fb + 1 : 2 * fb + 2],
                            a_sb, ALU.add, ALU.mult)
                        nc.sync.dma_start(
                            m_f[fb * 128 : (fb + 1) * 128,
                                sc * SCH : (sc + 1) * SCH],
                            m_sb.rearrange("p a b -> p (a b)"))

            # ---------------- Phase A+O: attention + output proj + RS ---------
            with (
                tc.tile_pool(name="wo", bufs=1) as wo,
                tc.tile_pool(name="attk", bufs=1) as attk,
                tc.tile_pool(name="attv", bufs=1) as attv,
                tc.tile_pool(name="qtp", bufs=2) as qtp,
                tc.tile_pool(name="mop", bufs=2) as mop,
                tc.tile_pool(name="ptp", bufs=4) as ptp,
                tc.tile_pool(name="atts", bufs=2) as atts,
                tc.tile_pool(name="pop", bufs=3) as pop,
                tc.tile_pool(name="eop", bufs=1) as eop,
                tc.tile_pool(name="psS", bufs=2, space="PSUM") as psS,
                tc.tile_pool(name="psD", bufs=1, space="PSUM") as psD,
                tc.tile_pool(name="psV", bufs=1, space="PSUM") as psV,
                tc.tile_pool(name="psO", bufs=2, space="PSUM") as psO,
            ):
                # kT via XBAR transpose of kr_d; v/wout plain loads
                kT_sb = attk.tile([128, HL, S], BF16)
                nc.scalar.dma_start_transpose(kT_sb, kr_d)
                v_sb = attv.tile([128, NKT, FQ], BF16)
                nc.scalar.dma_start(v_sb, v_d.rearrange("(t p) f -> p t f", p=128))

                qts, mts = {}, {}

                def load_qc(qc):
                    q0 = qc * QC
                    qt = qtp.tile([128, HL, QC], BF16, tag="qt", name="qt")
                    nc.scalar.dma_start_transpose(qt, qr_d[q0 : q0 + QC, :])
                    mt = mop.tile([128, FMT, QC], BF16, tag="mt", name="mt")
                    nc.scalar.dma_start(
                        mt, m_f.rearrange("(t p) s -> p t s", p=128)[
                            :, :, q0 : q0 + QC])
                    qts[qc], mts[qc] = qt, mt

                load_qc(0)
                wo_sb = wo.tile([128, WOT, D], BF16)
                nc.scalar.dma_start(wo_sb, wout.rearrange("(kt p) n -> p kt n", p=128))

                def epilogue(ri):
                    c0 = ri * RSW
                    rsb = eop.tile([128, HL, RSW], BF16, tag="rsb", name="rsb")
                    nc.sync.dma_start(
                        rsb, rs_out[ri].rearrange("(t p) s -> p t s", p=128))
                    rt = eop.tile([128, HL, RSW], F32, tag="rt", name="rt")
                    nc.sync.dma_start(
                        rt, resT.rearrange("(t p) s -> p t s", p=128)[
                            :, :, c0 : c0 + RSW])
                    ot = eop.tile([128, HL, RSW], F32, tag="ot", name="ot")
                    nc.vector.tensor_add(out=ot, in0=rt, in1=rsb)
                    nc.sync.dma_start(
                        out_t.rearrange("(t p) s -> p t s", p=128)[
                            :, :, c0 : c0 + RSW],
                        ot)

                for qc in range(NQC):
                    ri, rc = divmod(qc, SUBRS)
                    if qc + 1 < NQC:
                        load_qc(qc + 1)
                    qt, mt = qts.pop(qc), mts.pop(qc)
                    aos = []
                    for h in range(HL):
                        pden = psD.tile([128, QC], F32, tag="pden", name="pden")
                        pacc = psV.tile([128, QC], F32, tag="pacc", name="pacc")
                        pend = []

                        def drain(pden=pden, pacc=pacc, h=h):
                            pt, k2 = pend.pop(0)
                            for kk in range(2):
                                ki = k2 * 2 + kk
                                nc.tensor.matmul(
                                    pden, ones128, pt[:, kk, :],
                                    start=(ki == 0), stop=(ki == NKT - 1))
                                nc.tensor.matmul(
                                    pacc, v_sb[:, ki, h * HD : (h + 1) * HD],
                                    pt[:, kk, :],
                                    start=(ki == 0), stop=(ki == NKT - 1))

                        for k2 in range(NKT // 2):
                            pss = psS.tile([128, 2, QC], F32, tag="pss", name="pss")
                            for kk in range(2):
                                ki = k2 * 2 + kk
                                nc.tensor.matmul(
                                    pss[:, kk, :],
                                    kT_sb[:, h, ki * 128 : (ki + 1) * 128],
                                    qt[:, h, :], start=True, stop=True)
                            if len(pend) == 2:
                                drain()
                            pt = ptp.tile([128, 2, QC], BF16, tag="pt", name="pt")
                            nc.scalar.activation(pt, pss, AF.Exp)
                            pend.append((pt, k2))
                        while pend:
                            drain()
                        invd = atts.tile([128, QC], F32, tag="invd", name="invd")
                        nc.vector.reciprocal(invd, pden)
                        ao = atts.tile([128, QC], BF16, tag=f"ao{h}", name=f"ao{h}")
                        nc.vector.tensor_mul(out=ao, in0=pacc, in1=invd)
                        aos.append(ao)
                    # output projection partial: [D, QC] = woutT @ [attn; mlp]
                    for dt in range(KT):
                        po = psO.tile([128, QC], F32, tag="po", name="po")
                        for t in range(WOT):
                            rhs = aos[t] if t < HL else mt[:, t - HL, :]
                            nc.tensor.matmul(
                                po, wo_sb[:, t, dt * 128 : (dt + 1) * 128], rhs,
                                start=(t == 0), stop=(t == WOT - 1))
                        pout = pop.tile([128, QC], BF16, tag="pout", name="pout")
                        if dt % 2 == 0:
                            nc.scalar.copy(pout, po)
                        else:
                            nc.vector.tensor_copy(out=pout, in_=po)
                        nc.sync.dma_start(
                            pall[ri][dt * 128 : (dt + 1) * 128,
                                     rc * QC : (rc + 1) * QC],
                            pout)
                    if rc == SUBRS - 1:
                        nc.gpsimd.collective_compute(
                            "ReduceScatter", ALU.add, replica_groups=rg,
                            ins=[pall[ri].opt()], outs=[rs_out[ri].opt()])
                        if ri >= 2:
                            epilogue(ri - 2)
                epilogue(NRS - 2)
                epilogue(NRS - 1)

    nc.finalize()
    return nc


def _prep(inputs):
    hs = np.asarray(inputs["hidden_states"], np.float32).reshape(S, D)
    temb = np.asarray(inputs["temb_mod"], np.float32).reshape(3 * D)
    shift, scale, gate = temb[:D], temb[D : 2 * D], temb[2 * D :]
    cos = np.asarray(inputs["rotary_cos"], np.float32)
    sin = np.asarray(inputs["rotary_sin"], np.float32)
    w1 = np.asarray(inputs["w_qkv_mlp"], np.float32)
    wa = np.asarray(inputs["w_out_attn"], np.float32)
    wm = np.asarray(inputs["w_out_mlp"], np.float32)
    nqw = np.asarray(inputs["norm_q_w"], np.float32)
    nkw = np.asarray(inputs["norm_k_w"], np.float32)

    sgn = np.ones(HD, np.float32)
    sgn[0::2] = -1.0
    xb = _to_bf16(hs)
    cosb = _to_bf16(cos)
    sinb = _to_bf16(sin * sgn)
    alpha = float(HD) ** -0.25
    qwb = np.tile((nqw * alpha)[None, :], (128, 1)).astype(np.float32)
    kwb = np.tile((nkw * alpha)[None, :], (128, 1)).astype(np.float32)
    onep = (1.0 + scale)[:, None]

    in_maps = []
    for c in range(NCORES):
        q0, k0, v0 = c * FQ, D + c * FQ, 2 * D + c * FQ
        a0, b0 = 3 * D + c * FM, 3 * D + MLPH + c * FM
        w1qkv = np.concatenate(
            [w1[:, q0 : q0 + FQ], w1[:, k0 : k0 + FQ], w1[:, v0 : v0 + FQ]], axis=1)
        a_c = w1[:, a0 : a0 + FM].reshape(D, FMT, 128)
        b_c = w1[:, b0 : b0 + FM].reshape(D, FMT, 128)
        w1ab_c = np.stack([a_c, b_c], axis=2).reshape(D, FAB)
        c2q_c = (shift @ w1qkv)[None, :]
        c2ab_c = (shift @ w1ab_c).reshape(2 * FMT, 128).T
        n0 = c * NCOL
        wout_c = np.concatenate(
            [wa[c * FQ : (c + 1) * FQ, :], wm[c * FM : (c + 1) * FM, :]],
            axis=0) * gate[None, :]
        in_maps.append(dict(
            x=xb,
            w1qkv=_to_bf16(w1qkv * onep),
            w1ab=_to_bf16(w1ab_c * onep),
            c2q=_to_bf16(c2q_c),
            c2ab=np.ascontiguousarray(c2ab_c, np.float32),
            cosb=cosb, sinb=sinb, qwb=qwb, kwb=kwb,
            wout=_to_bf16(wout_c),
            resT=np.ascontiguousarray(hs[:, n0 : n0 + NCOL].T),
        ))
    return in_maps


def kernel(**inputs):
    global LAST_RESULT
    nc = _build()
    in_maps = _prep(inputs)
    r = run_bass_kernel_spmd(nc, in_maps, core_ids=list(range(NCORES)))
    LAST_RESULT = r
    full = np.concatenate([m["out"].T for m in r.results], axis=1)
    return full.reshape(1, S, D).astype(np.float32)


# revision 15
# speedup vs baseline: 1.0640x; 1.0640x over previous
import sys, functools

if "/opt/trn_rl_repo" not in sys.path:
    sys.path.insert(0, "/opt/trn_rl_repo")

import numpy as np
import ml_dtypes

from concourse import bacc
import concourse.bass as bass
import concourse.mybir as mybir
import concourse.tile as tile
from concourse.bass_utils import run_bass_kernel_spmd

BF16 = mybir.dt.bfloat16
F32 = mybir.dt.float32
AF = mybir.ActivationFunctionType
ALU = mybir.AluOpType
AX = mybir.AxisListType

S, D, HD, H, MLPH = 4096, 3072, 128, 24, 9216
NCORES = 8
HL = H // NCORES            # 3 heads per core
FQ = HL * HD                # 384
FM = MLPH // NCORES         # 1152
FMT = FM // 128             # 9 f-tiles of mlp hidden
FAB = 2 * FM                # 2304 (a/b interleaved in 128-col pairs)
FQKV = 3 * FQ               # 1152
NCOL = D // NCORES          # 384 output cols per core
FO = FQ + FM                # 1536 rows of fused output weight
WOT = FO // 128             # 12 contraction tiles of output proj
EPS = 1e-6
SCH = 1024                  # s-chunk for projection phase
NSC = S // SCH              # 4
KT = D // 128               # 24 contraction tiles of input proj
NKT = S // 128              # 32 k tiles in attention
QC = 512                    # q-chunk for attention/output phase
NQC = S // QC               # 8
NRS = 4                     # number of ReduceScatter chunks
RSW = S // NRS              # 1024 s-columns per RS chunk
SUBRS = NQC // NRS          # q-chunks per RS chunk

LAST_RESULT = None          # test.py introspection


def _to_bf16(a):
    """Fast round-to-nearest f32 -> bf16."""
    a = np.ascontiguousarray(a, np.float32)
    u = a.view(np.uint32)
    r = ((u >> 16) & 1) + np.uint32(0x7FFF)
    return ((u + r) >> 16).astype(np.uint16).view(ml_dtypes.bfloat16)


@functools.lru_cache(maxsize=1)
def _build():
    nc = bacc.Bacc(
        "TRN2",
        target_bir_lowering=False,
        debug=False,
        enable_asserts=False,
        num_devices=NCORES,
    )
    x = nc.dram_tensor("x", [S, D], BF16, kind="ExternalInput").ap()
    w1q = nc.dram_tensor("w1qkv", [D, FQKV], BF16, kind="ExternalInput").ap()
    w1ab = nc.dram_tensor("w1ab", [D, FAB], BF16, kind="ExternalInput").ap()
    c2q = nc.dram_tensor("c2q", [1, FQKV], BF16, kind="ExternalInput").ap()
    c2ab = nc.dram_tensor("c2ab", [128, 2 * FMT], F32, kind="ExternalInput").ap()
    cosb = nc.dram_tensor("cosb", [S, HD], BF16, kind="ExternalInput").ap()
    sinb = nc.dram_tensor("sinb", [S, HD], BF16, kind="ExternalInput").ap()
    qwb = nc.dram_tensor("qwb", [128, HD], F32, kind="ExternalInput").ap()
    kwb = nc.dram_tensor("kwb", [128, HD], F32, kind="ExternalInput").ap()
    wout = nc.dram_tensor("wout", [FO, D], BF16, kind="ExternalInput").ap()
    resT = nc.dram_tensor("resT", [NCOL, S], F32, kind="ExternalInput").ap()
    out_t = nc.dram_tensor("out", [NCOL, S], F32, kind="ExternalOutput").ap()

    rg = [list(range(NCORES))]

    with tile.TileContext(nc) as tc:
        with (
            tc.tile_pool(name="const", bufs=1) as const,
            tc.tile_pool(name="attk", bufs=1) as attk,
            tc.tile_pool(name="dram", bufs=1, space="DRAM") as dram,
        ):
            kT_sb = attk.tile([128, HL, S], BF16)
            ones_p = const.tile([1, 128], BF16)
            nc.vector.memset(ones_p, 1.0)
            ones128 = const.tile([128, 128], BF16)
            nc.vector.memset(ones128, 1.0)
            eps_sb = const.tile([128, 1], F32)
            nc.vector.memset(eps_sb, EPS)
            qwb_sb = const.tile([128, HD], F32)
            nc.sync.dma_start(qwb_sb, qwb)
            kwb_sb = const.tile([128, HD], F32)
            nc.sync.dma_start(kwb_sb, kwb)
            c2q_sb = const.tile([1, FQKV], BF16)
            nc.sync.dma_start(c2q_sb, c2q)
            c2ab_sb = const.tile([128, 2 * FMT], F32)
            nc.sync.dma_start(c2ab_sb, c2ab)

            tti_d = dram.tile([S, D], BF16)     # normalized x, s-major
            qr_d = dram.tile([S, FQ], BF16)     # rope(q), s-major
            kr_d = dram.tile([S, FQ], BF16)
            v_d = dram.tile([S, FQ], BF16)
            m_f = dram.tile([FM, S], BF16)      # swiglu output, f-major
            pall = [dram.tile([D, RSW], BF16, tag=f"pall{i}", name=f"pall{i}")
                    for i in range(NRS)]
            rs_out = [dram.tile([NCOL, RSW], BF16, tag=f"rso{i}", name=f"rso{i}")
                      for i in range(NRS)]

            # ---------------- Phase P: LN + transpose + QKV/MLP projection ----
            with (
                tc.tile_pool(name="w1qp", bufs=1) as w1qp,
                tc.tile_pool(name="xp", bufs=2) as xp,
                tc.tile_pool(name="tp", bufs=2) as tp,
                tc.tile_pool(name="ttp", bufs=1) as ttp,
                tc.tile_pool(name="w1s", bufs=2) as w1s,
                tc.tile_pool(name="smal", bufs=12) as smal,
                tc.tile_pool(name="stg", bufs=2) as stg,
                tc.tile_pool(name="abp", bufs=3) as abp,
                tc.tile_pool(name="psab", bufs=1, space="PSUM") as psab,
                tc.tile_pool(name="psq", bufs=4, space="PSUM") as psq,
            ):
                # resident QKV weight: [dm_part, kt, 1152]; loaded in 4 pieces
                # interleaved with the first LN tiles to not block the x loads
                w1q_sb = w1qp.tile([128, KT, FQKV], BF16)
                w1q_r = w1q.rearrange("(kt p) f -> p kt f", p=128)

                for sc in range(NSC):
                    tT = ttp.tile([128, KT, SCH], BF16, tag="tT")
                    for ss in range(8):
                        s0 = sc * SCH + ss * 128
                        xt = xp.tile([128, D], BF16, tag="x")
                        nc.sync.dma_start(xt, x[s0 : s0 + 128, :])
                        if sc == 0 and ss < 4:
                            k6 = KT // 4
                            nc.sync.dma_start(
                                w1q_sb[:, ss * k6 : (ss + 1) * k6, :],
                                w1q_r[:, ss * k6 : (ss + 1) * k6, :])
                        s1 = smal.tile([128, 1], F32, tag="s1")
                        nc.vector.reduce_sum(s1, xt, axis=AX.X)
                        nmu = smal.tile([128, 1], F32, tag="nmu")
                        nc.scalar.mul(nmu, s1, -1.0 / D)
                        sqs = tp.tile([128, D], BF16, tag="sq", bufs=1)
                        v2 = smal.tile([128, 1], F32, tag="v2")
                        nc.scalar.activation(sqs, xt, AF.Square, bias=nmu, scale=1.0,
                                             accum_out=v2)
                        std = smal.tile([128, 1], F32, tag="std")
                        nc.scalar.activation(std, v2, AF.Sqrt, bias=eps_sb, scale=1.0 / D)
                        rstd = smal.tile([128, 1], F32, tag="rstd")
                        nc.vector.reciprocal(rstd, std)
                        nmr = smal.tile([128, 1], F32, tag="nmr")
                        nc.vector.tensor_mul(out=nmr, in0=nmu, in1=rstd)
                        tti = tp.tile([128, D], BF16, tag="t")
                        nc.scalar.activation(tti, xt, AF.Identity, bias=nmr, scale=rstd)
                        # round-trip through DRAM + XBAR dma transpose -> tT
                        nc.sync.dma_start(tti_d[s0 : s0 + 128, :], tti)
                        nc.sync.dma_start_transpose(
                            tT[:, :, ss * 128 : (ss + 1) * 128],
                            tti_d[s0 : s0 + 128, :])

                    # --- QKV pass (s-major): one psum bank per j ---
                    for ss in range(8):
                        s0 = sc * SCH + ss * 128
                        sl = ss * 128
                        cos_t = stg.tile([128, HD], BF16, tag="cos")
                        nc.sync.dma_start(cos_t, cosb[s0 : s0 + 128, :])
                        sin_t = stg.tile([128, HD], BF16, tag="sin")
                        nc.sync.dma_start(sin_t, sinb[s0 : s0 + 128, :])
                        s2_ = sin_t.rearrange("p (x two) -> p x two", two=2)
                        for j in range(3):
                            pq = psq.tile([128, 512], F32, tag="pqkv")
                            for kt in range(KT):
                                nc.tensor.matmul(
                                    pq[:, :FQ],
                                    tT[:, kt, sl : sl + 128],
                                    w1q_sb[:, kt, j * FQ : (j + 1) * FQ],
                                    start=(kt == 0), stop=False)
                            nc.tensor.matmul(
                                pq[:, :FQ], ones_p,
                                c2q_sb[:, j * FQ : (j + 1) * FQ],
                                start=False, stop=True)
                            if j == 2:
                                vstg = stg.tile([128, FQ], BF16, tag="vst", bufs=2)
                                nc.scalar.copy(vstg, pq[:, :FQ])
                                nc.sync.dma_start(v_d[s0 : s0 + 128, :], vstg)
                                continue
                            wb = qwb_sb if j == 0 else kwb_sb
                            qn = stg.tile([128, FQ], BF16, tag=f"qn{j}")
                            qrr = stg.tile([128, FQ], BF16, tag=f"qr{j}")
                            tmp = stg.tile([128, FQ], BF16, tag=f"tm{j}")
                            for hh in range(HL):
                                blk = pq[:, hh * HD : (hh + 1) * HD]
                                ssq = smal.tile([128, 1], F32, tag="ssq")
                                sq2 = stg.tile([128, HD], F32, tag="sq2")
                                nc.scalar.activation(sq2, blk, AF.Square, accum_out=ssq)
                                sstd = smal.tile([128, 1], F32, tag="sstd")
                                nc.scalar.activation(sstd, ssq, AF.Sqrt,
                                                     bias=eps_sb, scale=1.0 / HD)
                                rst = smal.tile([128, 1], F32, tag="rst")
                                nc.vector.reciprocal(rst, sstd)
                                qnb = qn[:, hh * HD : (hh + 1) * HD]
                                nc.vector.scalar_tensor_tensor(
                                    qnb, blk, rst, wb, ALU.mult, ALU.mult)
                                q3 = qnb.rearrange("p (x two) -> p x two", two=2)
                                t3 = tmp[:, hh * HD : (hh + 1) * HD].rearrange(
                                    "p (x two) -> p x two", two=2)
                                nc.vector.tensor_mul(out=t3[:, :, 0], in0=q3[:, :, 1],
                                                     in1=s2_[:, :, 0])
                                nc.vector.tensor_mul(out=t3[:, :, 1], in0=q3[:, :, 0],
                                                     in1=s2_[:, :, 1])
                                nc.vector.tensor_mul(
                                    out=qrr[:, hh * HD : (hh + 1) * HD],
                                    in0=qnb, in1=cos_t)
                            nc.vector.tensor_add(out=qrr, in0=qrr, in1=tmp)
                            dst = qr_d if j == 0 else kr_d
                            nc.sync.dma_start(dst[s0 : s0 + 128, :], qrr)

                    if sc == NSC - 1:
                        # k fully written: start the big attention-K transpose
                        # so it overlaps the last a/b pass
                        nc.sync.dma_start_transpose(kT_sb, kr_d)

                    # --- a/b (f-major) + SwiGLU ---
                    for fb in range(FMT):
                        pa = psab.tile([128, 2, 2, 512], F32, tag="pab")
                        for kg in range(KT // 8):
                            wt = w1s.tile([128, 8, 256], BF16, tag="w1ab")
                            nc.sync.dma_start(
                                wt, w1ab.rearrange("(kt p) f -> p kt f", p=128)[
                                    :, kg * 8 : (kg + 1) * 8,
                                    fb * 256 : (fb + 1) * 256])
                            for k8 in range(8):
                                kt = kg * 8 + k8
                                for f2 in range(2):
                                    for sh in range(2):
                                        nc.tensor.matmul(
                                            pa[:, f2, sh, :],
                                            wt[:, k8, f2 * 128 : (f2 + 1) * 128],
                                            tT[:, kt, sh * 512 : (sh + 1) * 512],
                                            start=(kt == 0), stop=(kt == KT - 1))
                        a_sb = abp.tile([128, 2, 512], BF16, tag="asb")
                        nc.scalar.activation(a_sb, pa[:, 0], AF.Silu,
                                             bias=c2ab_sb[:, 2 * fb : 2 * fb + 1])
                        m_sb = abp.tile([128, 2, 512], BF16, tag="msb")
                        nc.vector.scalar_tensor_tensor(
                            m_sb, pa[:, 1], c2ab_sb[:, 2 * fb + 1 : 2 * fb + 2],
                            a_sb, ALU.add, ALU.mult)
                        nc.sync.dma_start(
                            m_f[fb * 128 : (fb + 1) * 128,
                                sc * SCH : (sc + 1) * SCH],
                            m_sb.rearrange("p a b -> p (a b)"))

            # ---------------- Phase A+O: attention + output proj + RS ---------
            with (
                tc.tile_pool(name="wo", bufs=1) as wo,
                tc.tile_pool(name="attv", bufs=1) as attv,
                tc.tile_pool(name="qtp", bufs=2) as qtp,
                tc.tile_pool(name="mop", bufs=2) as mop,
                tc.tile_pool(name="ptp", bufs=4) as ptp,
                tc.tile_pool(name="atts", bufs=2) as atts,
                tc.tile_pool(name="pop", bufs=3) as pop,
                tc.tile_pool(name="eop", bufs=1) as eop,
                tc.tile_pool(name="psS", bufs=2, space="PSUM") as psS,
                tc.tile_pool(name="psD", bufs=1, space="PSUM") as psD,
                tc.tile_pool(name="psV", bufs=1, space="PSUM") as psV,
                tc.tile_pool(name="psO", bufs=2, space="PSUM") as psO,
            ):
                v_sb = attv.tile([128, NKT, FQ], BF16)
                nc.sync.dma_start(v_sb, v_d.rearrange("(t p) f -> p t f", p=128))

                qts, mts = {}, {}

                def load_qc(qc):
                    q0 = qc * QC
                    qt = qtp.tile([128, HL, QC], BF16, tag="qt", name="qt")
                    nc.sync.dma_start_transpose(qt, qr_d[q0 : q0 + QC, :])
                    mt = mop.tile([128, FMT, QC], BF16, tag="mt", name="mt")
                    nc.sync.dma_start(
                        mt, m_f.rearrange("(t p) s -> p t s", p=128)[
                            :, :, q0 : q0 + QC])
                    qts[qc], mts[qc] = qt, mt

                load_qc(0)
                wo_sb = wo.tile([128, WOT, D], BF16)
                nc.sync.dma_start(wo_sb, wout.rearrange("(kt p) n -> p kt n", p=128))

                def epilogue(ri):
                    c0 = ri * RSW
                    rsb = eop.tile([128, HL, RSW], BF16, tag="rsb", name="rsb")
                    nc.sync.dma_start(
                        rsb, rs_out[ri].rearrange("(t p) s -> p t s", p=128))
                    rt = eop.tile([128, HL, RSW], F32, tag="rt", name="rt")
                    nc.sync.dma_start(
                        rt, resT.rearrange("(t p) s -> p t s", p=128)[
                            :, :, c0 : c0 + RSW])
                    ot = eop.tile([128, HL, RSW], F32, tag="ot", name="ot")
                    nc.vector.tensor_add(out=ot, in0=rt, in1=rsb)
                    nc.sync.dma_start(
                        out_t.rearrange("(t p) s -> p t s", p=128)[
                            :, :, c0 : c0 + RSW],
                        ot)

                for qc in range(NQC):
                    ri, rc = divmod(qc, SUBRS)
                    if qc + 1 < NQC:
                        load_qc(qc + 1)
                    qt, mt = qts.pop(qc), mts.pop(qc)
                    aos = []
                    for h in range(HL):
                        pden = psD.tile([128, QC], F32, tag="pden", name="pden")
                        pacc = psV.tile([128, QC], F32, tag="pacc", name="pacc")
                        pend = []

                        def drain(pden=pden, pacc=pacc, h=h):
                            pt, k2 = pend.pop(0)
                            for kk in range(2):
                                ki = k2 * 2 + kk
                                nc.tensor.matmul(
                                    pden, ones128, pt[:, kk, :],
                                    start=(ki == 0), stop=(ki == NKT - 1))
                                nc.tensor.matmul(
                                    pacc, v_sb[:, ki, h * HD : (h + 1) * HD],
                                    pt[:, kk, :],
                                    start=(ki == 0), stop=(ki == NKT - 1))

                        for k2 in range(NKT // 2):
                            pss = psS.tile([128, 2, QC], F32, tag="pss", name="pss")
                            for kk in range(2):
                                ki = k2 * 2 + kk
                                nc.tensor.matmul(
                                    pss[:, kk, :],
                                    kT_sb[:, h, ki * 128 : (ki + 1) * 128],
                                    qt[:, h, :], start=True, stop=True)
                            if len(pend) == 2:
                                drain()
                            pt = ptp.tile([128, 2, QC], BF16, tag="pt", name="pt")
                            nc.scalar.activation(pt, pss, AF.Exp)
                            pend.append((pt, k2))
                        while pend:
                            drain()
                        invd = atts.tile([128, QC], F32, tag="invd", name="invd")
                        nc.vector.reciprocal(invd, pden)
                        ao = atts.tile([128, QC], BF16, tag=f"ao{h}", name=f"ao{h}")
                        nc.vector.tensor_mul(out=ao, in0=pacc, in1=invd)
                        aos.append(ao)
                    # output projection partial: [D, QC] = woutT @ [attn; mlp]
                    for dt in range(KT):
                        po = psO.tile([128, QC], F32, tag="po", name="po")
                        for t in range(WOT):
                            rhs = aos[t] if t < HL else mt[:, t - HL, :]
                            nc.tensor.matmul(
                                po, wo_sb[:, t, dt * 128 : (dt + 1) * 128], rhs,
                                start=(t == 0), stop=(t == WOT - 1))
                        pout = pop.tile([128, QC], BF16, tag="pout", name="pout")
                        if dt % 2 == 0:
                            nc.scalar.copy(pout, po)
                        else:
                            nc.vector.tensor_copy(out=pout, in_=po)
                        nc.sync.dma_start(
                            pall[ri][dt * 128 : (dt + 1) * 128,
                                     rc * QC : (rc + 1) * QC],
                            pout)
                    if rc == SUBRS - 1:
                        nc.gpsimd.collective_compute(
                            "ReduceScatter", ALU.add, replica_groups=rg,
                            ins=[pall[ri].opt()], outs=[rs_out[ri].opt()])
                for ri in range(NRS):
                    epilogue(ri)

    nc.finalize()
    return nc


def _prep(inputs):
    hs = np.asarray(inputs["hidden_states"], np.float32).reshape(S, D)
    temb = np.asarray(inputs["temb_mod"], np.float32).reshape(3 * D)
    shift, scale, gate = temb[:D], temb[D : 2 * D], temb[2 * D :]
    cos = np.asarray(inputs["rotary_cos"], np.float32)
    sin = np.asarray(inputs["rotary_sin"], np.float32)
    w1 = np.asarray(inputs["w_qkv_mlp"], np.float32)
    wa = np.asarray(inputs["w_out_attn"], np.float32)
    wm = np.asarray(inputs["w_out_mlp"], np.float32)
    nqw = np.asarray(inputs["norm_q_w"], np.float32)
    nkw = np.asarray(inputs["norm_k_w"], np.float32)

    sgn = np.ones(HD, np.float32)
    sgn[0::2] = -1.0
    xb = _to_bf16(hs)
    cosb = _to_bf16(cos)
    sinb = _to_bf16(sin * sgn)
    alpha = float(HD) ** -0.25
    qwb = np.tile((nqw * alpha)[None, :], (128, 1)).astype(np.float32)
    kwb = np.tile((nkw * alpha)[None, :], (128, 1)).astype(np.float32)
    onep = (1.0 + scale)[:, None]

    in_maps = []
    for c in range(NCORES):
        q0, k0, v0 = c * FQ, D + c * FQ, 2 * D + c * FQ
        a0, b0 = 3 * D + c * FM, 3 * D + MLPH + c * FM
        w1qkv = np.concatenate(
            [w1[:, q0 : q0 + FQ], w1[:, k0 : k0 + FQ], w1[:, v0 : v0 + FQ]], axis=1)
        a_c = w1[:, a0 : a0 + FM].reshape(D, FMT, 128)
        b_c = w1[:, b0 : b0 + FM].reshape(D, FMT, 128)
        w1ab_c = np.stack([a_c, b_c], axis=2).reshape(D, FAB)
        c2q_c = (shift @ w1qkv)[None, :]
        c2ab_c = (shift @ w1ab_c).reshape(2 * FMT, 128).T
        n0 = c * NCOL
        wout_c = np.concatenate(
            [wa[c * FQ : (c + 1) * FQ, :], wm[c * FM : (c + 1) * FM, :]],
            axis=0) * gate[None, :]
        in_maps.append(dict(
            x=xb,
            w1qkv=_to_bf16(w1qkv * onep),
            w1ab=_to_bf16(w1ab_c * onep),
            c2q=_to_bf16(c2q_c),
            c2ab=np.ascontiguousarray(c2ab_c, np.float32),
            cosb=cosb, sinb=sinb, qwb=qwb, kwb=kwb,
            wout=_to_bf16(wout_c),
            resT=np.ascontiguousarray(hs[:, n0 : n0 + NCOL].T),
        ))
    return in_maps


def kernel(**inputs):
    global LAST_RESULT
    nc = _build()
    in_maps = _prep(inputs)
    r = run_bass_kernel_spmd(nc, in_maps, core_ids=list(range(NCORES)))
    LAST_RESULT = r
    full = np.concatenate([m["out"].T for m in r.results], axis=1)
    return full.reshape(1, S, D).astype(np.float32)
